# revision 1
# baseline (speedup 1.0000x reference)
"""AugmentedMamba3 kernel for 8 Trainium2 NeuronCores.

The reference's sequential scan is a *linear* recurrence in the state
(rank-1 writes, scalar decay), so it is re-expressed exactly as
causal-masked attention-style GEMMs — no per-step scan on device.

Sharding (8 cores): batch b in {0..3} x sequence-half hf in {0,1};
core i handles (b = i//2, hf = i%2), T = 1024 tokens.  Each core
computes the full-batch projections (cheap GEMMs) so that second-half
cores can form the incoming register/memory state from the first half
with two small GEMMs — no cross-core communication.  All cores run the
identical program (init terms are multiplied by an is_second flag).
"""

import numpy as np
import jax
import jax.numpy as jnp
from functools import partial

B, L, D = 4, 2048, 1024
T = L // 2
NREG, NMEM = 8, 16
DECAY = 0.995
SHARP = 5.0

_W_NAMES = [
    "reg_gate_w", "reg_gate_b", "reg_addr_w", "reg_addr_b", "reg_val_w",
    "reg_val_b", "reg_q_w", "reg_q_b", "mem_gate_w", "mem_gate_b",
    "mem_addr_w", "mem_addr_b", "mem_val_w", "mem_val_b", "mem_q_w",
    "mem_q_b", "comb_w", "comb_b", "ln_g", "ln_b",
]


def _shard_fn(u_all, hf_flags,
              reg_gate_w, reg_gate_b, reg_addr_w, reg_addr_b, reg_val_w,
              reg_val_b, reg_q_w, reg_q_b, mem_gate_w, mem_gate_b, mem_addr_w,
              mem_addr_b, mem_val_w, mem_val_b, mem_q_w, mem_q_b, comb_w,
              comb_b, ln_g, ln_b):
    # u_all: (L, D) full sequence of this shard's batch element
    # hf_flags: (2,) = (is_second, hf) as f32
    is2 = hf_flags[0]
    scale = D ** (-0.5)
    lin = lambda x, w, b: x @ w.T + b
    sm = jax.nn.softmax
    h_all = u_all

    rg = jax.nn.sigmoid(SHARP * lin(h_all, reg_gate_w, reg_gate_b))
    ra = sm(lin(h_all, reg_addr_w, reg_addr_b), axis=-1)
    rv = lin(h_all, reg_val_w, reg_val_b)
    mg = jax.nn.sigmoid(SHARP * lin(h_all, mem_gate_w, mem_gate_b))
    ma = sm(lin(h_all, mem_addr_w, mem_addr_b), axis=-1)
    mv = lin(h_all, mem_val_w, mem_val_b)
    A_r_all = rg * ra                                   # (L, NREG)
    A_m_all = mg * ma                                   # (L, NMEM)

    # own half (dynamic_slice with traced start keeps the program SPMD)
    s0 = (hf_flags[1] * T).astype(jnp.int32)
    sl = lambda x: jax.lax.dynamic_slice_in_dim(x, s0, T, axis=0)
    h = sl(h_all)
    rvh = sl(rv)
    mvh = sl(mv)
    A_r = sl(A_r_all)
    A_m = sl(A_m_all)
    # q projections only needed for own half
    rqh = lin(h, reg_q_w, reg_q_b)
    mqh = lin(h, mem_q_w, mem_q_b)

    W_h, W_r, W_m = comb_w[:, :D], comb_w[:, D:2 * D], comb_w[:, 2 * D:]
    pre = h @ W_h.T + comb_b

    t = jnp.arange(T, dtype=jnp.float32)
    causal = (t[:, None] >= t[None, :]).astype(jnp.float32)
    dpow = jnp.exp((t + 1.0) * np.log(DECAY))[:, None]   # DECAY^(t+1)
    # decay^(t-s) folded into row scalings (avoids a T x T dmask tensor):
    dp = jnp.exp(t * np.log(DECAY))[:, None]             # DECAY^t
    dm = jnp.exp(-t * np.log(DECAY))[:, None]            # DECAY^-s

    # incoming state from the first half (zeroed on first-half shards)
    Ar_p, rv_p = A_r_all[:T], rv[:T]
    Am_p, mv_p = A_m_all[:T], mv[:T]
    wdec = jnp.exp((T - 1.0 - t) * np.log(DECAY))[:, None]
    reg_init = is2 * (Ar_p.T @ rv_p)                     # (NREG, D)
    mem_init = is2 * ((Am_p * wdec).T @ mv_p)            # (NMEM, D)

    # register bank
    G = rqh @ rvh.T                                      # (T, T)
    scores = scale * ((G * causal) @ A_r + rqh @ reg_init.T)
    P = sm(scores, axis=-1)
    R = ((P @ A_r.T) * causal) @ rvh + P @ reg_init      # (T, D)

    # persistent memory: (Gm . dmask) == ((mq*dp) @ (mv*dm).T) . causal
    Gm = (mqh * dp) @ (mvh * dm).T
    scores_m = scale * ((Gm * causal) @ A_m + dpow * (mqh @ mem_init.T))
    Pm = sm(scores_m, axis=-1)
    M = dp * (((Pm @ (A_m * dm).T) * causal) @ mvh) + dpow * (Pm @ mem_init)

    c = pre + R @ W_r.T + M @ W_m.T + h
    mu = c.mean(-1, keepdims=True)
    v = ((c - mu) ** 2).mean(-1, keepdims=True)
    return ln_g * (c - mu) * jax.lax.rsqrt(v + 1e-5) + ln_b


_pmapped = None


def _get_pmapped():
    global _pmapped
    if _pmapped is None:
        _pmapped = jax.pmap(
            _shard_fn,
            in_axes=(0, 0) + (None,) * len(_W_NAMES),
            devices=jax.devices()[:8],
        )
    return _pmapped


def kernel(**inputs):
    u = np.asarray(inputs["u"], dtype=np.float32)
    ws = [np.asarray(inputs[k], dtype=np.float32) for k in _W_NAMES]

    # shard i -> (batch i//2, half i%2); each shard carries the full batch seq
    u_sh = np.stack([u[i // 2] for i in range(8)])               # (8, L, D)
    hf = np.array([[float(i % 2), float(i % 2)] for i in range(8)],
                  dtype=np.float32)                              # (8, 2)

    out_sh = _get_pmapped()(u_sh, hf, *ws)                       # (8, T, D)
    out_sh = np.asarray(jax.device_get(out_sh), dtype=np.float32)

    out = np.empty((B, L, D), dtype=np.float32)
    for b in range(B):
        out[b, :T] = out_sh[2 * b]
        out[b, T:] = out_sh[2 * b + 1]
    return out


if __name__ == "__main__":
    rng = np.random.default_rng(0)
    demo = {"u": rng.standard_normal((B, L, D)).astype(np.float32)}
    # minimal smoke with zero weights
    for k in _W_NAMES:
        demo[k] = None
    print("import ok")



# revision 2
# speedup vs baseline: 11.4323x; 11.4323x over previous
"""AugmentedMamba3 kernel for 8 Trainium2 NeuronCores.

The reference's sequential scan is a *linear* recurrence in the state
(rank-1 writes, scalar decay), so it is re-expressed exactly as
causal-masked attention-style GEMMs — no per-step scan on device.

Sharding (8 cores): batch b in {0..3} x sequence-half hf in {0,1};
core i handles (b = i//2, hf = i%2), T = 1024 tokens.  Second-half
cores form the incoming register/memory state from the first half via
folded summaries ((A^T h) W^T — tiny GEMMs instead of full first-half
value projections), so there is no cross-core communication.  All
cores run the identical program (init terms are multiplied by an
is_second flag).

Performance notes (vs the f32 XLA baseline at ~2.7 ms device time):
  - all O(T*D^2) / O(T^2*D) GEMMs run in bf16 with f32 accumulation
    (PE fast-weight-load needs non-f32; f32 matmul is ~4x slower),
  - first-half value projections folded into (8|16, D) summaries,
  - neuronx-cc flags `--optlevel 3 --model-type transformer` (the
    combination gives ~2x over either alone on this program).
Measured device time: ~450 us/iter (slope method, on-device repeats).
"""

import os

# Must be set before the first XLA->neuronx-cc compilation triggered below.
_FLAGS = "--optlevel 3 --model-type transformer"
_cur = os.environ.get("NEURON_CC_FLAGS", "--retry_failed_compilation")
if "--optlevel" not in _cur:
    os.environ["NEURON_CC_FLAGS"] = (_cur + " " + _FLAGS).strip()

import numpy as np
import jax
import jax.numpy as jnp

B, L, D = 4, 2048, 1024
T = L // 2
NREG, NMEM = 8, 16
DECAY = 0.995
SHARP = 5.0
BF = jnp.bfloat16
F32 = jnp.float32

_W_NAMES = [
    "reg_gate_w", "reg_gate_b", "reg_addr_w", "reg_addr_b", "reg_val_w",
    "reg_val_b", "reg_q_w", "reg_q_b", "mem_gate_w", "mem_gate_b",
    "mem_addr_w", "mem_addr_b", "mem_val_w", "mem_val_b", "mem_q_w",
    "mem_q_b", "comb_w", "comb_b", "ln_g", "ln_b",
]


def _mm(a, b):
    # bf16 x bf16 -> f32-accumulated matmul on the PE array
    return jax.lax.dot_general(
        a.astype(BF), b.astype(BF),
        (((a.ndim - 1,), (0,)), ((), ())),
        preferred_element_type=F32)


def _shard_fn(u_all, hf_flags,
              reg_gate_w, reg_gate_b, reg_addr_w, reg_addr_b, reg_val_w,
              reg_val_b, reg_q_w, reg_q_b, mem_gate_w, mem_gate_b, mem_addr_w,
              mem_addr_b, mem_val_w, mem_val_b, mem_q_w, mem_q_b, comb_w,
              comb_b, ln_g, ln_b):
    # u_all: (L, D) full sequence of this shard's batch element
    # hf_flags: (2,) = (is_second, hf) as f32
    is2 = hf_flags[0]
    scale = D ** (-0.5)
    sm = jax.nn.softmax
    h_all = u_all

    # gate + address projections over the full sequence (D -> 26, cheap)
    Wsm = jnp.concatenate([reg_gate_w, reg_addr_w, mem_gate_w, mem_addr_w], 0)
    bsm = jnp.concatenate([reg_gate_b, reg_addr_b, mem_gate_b, mem_addr_b], 0)
    sm_all = _mm(h_all, Wsm.T) + bsm                      # (L, 26) f32
    rg = jax.nn.sigmoid(SHARP * sm_all[:, 0:1])
    ra = sm(sm_all[:, 1:1 + NREG], axis=-1)
    mg = jax.nn.sigmoid(SHARP * sm_all[:, 1 + NREG:2 + NREG])
    ma = sm(sm_all[:, 2 + NREG:], axis=-1)
    A_r_all = rg * ra                                     # (L, NREG)
    A_m_all = mg * ma                                     # (L, NMEM)

    t = jnp.arange(T, dtype=F32)
    logd = np.log(DECAY).astype(np.float32)
    wdecT = jnp.exp((T - 1.0 - t) * logd)[:, None]        # DECAY^(T-1-t)

    # folded first-half summaries (zeroed on first-half shards):
    #   reg_init = (A_r[:T]^T h[:T]) Wv^T + (sum A_r) (x) bv
    Ar_p = A_r_all[:T]
    Am_pw = A_m_all[:T] * wdecT
    S_r = _mm(Ar_p.T, h_all[:T])                          # (NREG, D)
    S_m = _mm(Am_pw.T, h_all[:T])                         # (NMEM, D)
    reg_init = is2 * (_mm(S_r, reg_val_w.T)
                      + Ar_p.sum(0)[:, None] * reg_val_b[None, :])
    mem_init = is2 * (_mm(S_m, mem_val_w.T)
                      + Am_pw.sum(0)[:, None] * mem_val_b[None, :])

    # own half (dynamic_slice with traced start keeps the program SPMD)
    s0 = (hf_flags[1] * T).astype(jnp.int32)
    sl = lambda x: jax.lax.dynamic_slice_in_dim(x, s0, T, axis=0)
    h = sl(h_all)                                         # (T, D)
    A_r = sl(A_r_all)
    A_m = sl(A_m_all)
    rvh = _mm(h, reg_val_w.T) + reg_val_b
    mvh = _mm(h, mem_val_w.T) + mem_val_b
    rqh = _mm(h, reg_q_w.T) + reg_q_b
    mqh = _mm(h, mem_q_w.T) + mem_q_b

    W_h, W_r, W_m = comb_w[:, :D], comb_w[:, D:2 * D], comb_w[:, 2 * D:]
    pre = _mm(h, W_h.T) + comb_b

    causal = (t[:, None] >= t[None, :]).astype(F32)
    dpow = jnp.exp((t + 1.0) * logd)[:, None]             # DECAY^(t+1)
    dp = jnp.exp(t * logd)[:, None]                       # DECAY^t
    dm = jnp.exp(-t * logd)[:, None]                      # DECAY^-s

    # register bank
    G = _mm(rqh, rvh.T)                                   # (T, T)
    scores = scale * (_mm(G * causal, A_r) + _mm(rqh, reg_init.T))
    P = sm(scores, axis=-1)                               # (T, NREG)
    R = _mm(_mm(P, A_r.T) * causal, rvh) + _mm(P, reg_init)

    # persistent memory: decay^(t-s) folded into row scalings
    Gm = _mm(mqh * dp, (mvh * dm).T)
    scores_m = scale * (_mm(Gm * causal, A_m) + dpow * _mm(mqh, mem_init.T))
    Pm = sm(scores_m, axis=-1)
    M = dp * _mm(_mm(Pm, (A_m * dm).T) * causal, mvh) + dpow * _mm(Pm, mem_init)

    c = pre + _mm(R, W_r.T) + _mm(M, W_m.T) + h
    mu = c.mean(-1, keepdims=True)
    v = ((c - mu) ** 2).mean(-1, keepdims=True)
    return ln_g * (c - mu) * jax.lax.rsqrt(v + 1e-5) + ln_b


_pmapped = None


def _get_pmapped():
    global _pmapped
    if _pmapped is None:
        _pmapped = jax.pmap(
            _shard_fn,
            in_axes=(0, 0) + (None,) * len(_W_NAMES),
            devices=jax.devices()[:8],
        )
    return _pmapped


def kernel(**inputs):
    u = np.asarray(inputs["u"], dtype=np.float32)
    ws = [np.asarray(inputs[k], dtype=np.float32) for k in _W_NAMES]

    # shard i -> (batch i//2, half i%2); each shard carries the full batch seq
    u_sh = np.stack([u[i // 2] for i in range(8)])               # (8, L, D)
    hf = np.array([[float(i % 2), float(i % 2)] for i in range(8)],
                  dtype=np.float32)                              # (8, 2)

    out_sh = _get_pmapped()(u_sh, hf, *ws)                       # (8, T, D)
    out_sh = np.asarray(jax.device_get(out_sh), dtype=np.float32)

    out = np.empty((B, L, D), dtype=np.float32)
    for b in range(B):
        out[b, :T] = out_sh[2 * b]
        out[b, T:] = out_sh[2 * b + 1]
    return out


# revision 3
# speedup vs baseline: 12.7607x; 1.1162x over previous
"""AugmentedMamba3 kernel for 8 Trainium2 NeuronCores.

The reference's sequential scan is a *linear* recurrence in the state
(rank-1 writes, scalar decay), so it is re-expressed exactly as
causal-masked attention-style GEMMs — no per-step scan on device.

Sharding (8 cores): batch b in {0..3} x sequence-half hf in {0,1};
core i handles (b = i//2, hf = i%2), T = 1024 tokens.  Second-half
cores form the incoming register/memory state from the first half via
folded summaries ((A^T h) W^T — tiny GEMMs instead of full first-half
value projections), so there is no cross-core communication.  All
cores run the identical program (init terms are multiplied by an
is_second flag).

Performance notes (vs the f32 XLA baseline at ~2.7 ms device time):
  - all O(T*D^2) / O(T^2*D) GEMMs run in bf16 with f32 accumulation
    (PE fast-weight-load needs non-f32; f32 matmul is ~4x slower),
  - first-half value projections folded into (8|16, D) summaries
    ((A^T h) W^T instead of A^T (h W^T) — removes two full-length
    D x D GEMMs per core).
Measured device time: ~620 us/iter (slope method, on-device repeats;
neuronx-cc --optlevel/--model-type flags measured perf-neutral here).
"""

import numpy as np
import jax
import jax.numpy as jnp

B, L, D = 4, 2048, 1024
T = L // 2
NREG, NMEM = 8, 16
DECAY = 0.995
SHARP = 5.0
BF = jnp.bfloat16
F32 = jnp.float32

_W_NAMES = [
    "reg_gate_w", "reg_gate_b", "reg_addr_w", "reg_addr_b", "reg_val_w",
    "reg_val_b", "reg_q_w", "reg_q_b", "mem_gate_w", "mem_gate_b",
    "mem_addr_w", "mem_addr_b", "mem_val_w", "mem_val_b", "mem_q_w",
    "mem_q_b", "comb_w", "comb_b", "ln_g", "ln_b",
]


def _mm(a, b):
    # bf16 x bf16 -> f32-accumulated matmul on the PE array
    return jax.lax.dot_general(
        a.astype(BF), b.astype(BF),
        (((a.ndim - 1,), (0,)), ((), ())),
        preferred_element_type=F32)


def _shard_fn(u_all, hf_flags,
              reg_gate_w, reg_gate_b, reg_addr_w, reg_addr_b, reg_val_w,
              reg_val_b, reg_q_w, reg_q_b, mem_gate_w, mem_gate_b, mem_addr_w,
              mem_addr_b, mem_val_w, mem_val_b, mem_q_w, mem_q_b, comb_w,
              comb_b, ln_g, ln_b):
    # u_all: (L, D) full sequence of this shard's batch element
    # hf_flags: (2,) = (is_second, hf) as f32
    is2 = hf_flags[0]
    scale = D ** (-0.5)
    sm = jax.nn.softmax
    h_all = u_all

    # gate + address projections over the full sequence (D -> 26, cheap)
    Wsm = jnp.concatenate([reg_gate_w, reg_addr_w, mem_gate_w, mem_addr_w], 0)
    bsm = jnp.concatenate([reg_gate_b, reg_addr_b, mem_gate_b, mem_addr_b], 0)
    sm_all = _mm(h_all, Wsm.T) + bsm                      # (L, 26) f32
    rg = jax.nn.sigmoid(SHARP * sm_all[:, 0:1])
    ra = sm(sm_all[:, 1:1 + NREG], axis=-1)
    mg = jax.nn.sigmoid(SHARP * sm_all[:, 1 + NREG:2 + NREG])
    ma = sm(sm_all[:, 2 + NREG:], axis=-1)
    A_r_all = rg * ra                                     # (L, NREG)
    A_m_all = mg * ma                                     # (L, NMEM)

    t = jnp.arange(T, dtype=F32)
    logd = np.log(DECAY).astype(np.float32)
    wdecT = jnp.exp((T - 1.0 - t) * logd)[:, None]        # DECAY^(T-1-t)

    # folded first-half summaries (zeroed on first-half shards):
    #   reg_init = (A_r[:T]^T h[:T]) Wv^T + (sum A_r) (x) bv
    Ar_p = A_r_all[:T]
    Am_pw = A_m_all[:T] * wdecT
    S_r = _mm(Ar_p.T, h_all[:T])                          # (NREG, D)
    S_m = _mm(Am_pw.T, h_all[:T])                         # (NMEM, D)
    reg_init = is2 * (_mm(S_r, reg_val_w.T)
                      + Ar_p.sum(0)[:, None] * reg_val_b[None, :])
    mem_init = is2 * (_mm(S_m, mem_val_w.T)
                      + Am_pw.sum(0)[:, None] * mem_val_b[None, :])

    # own half (dynamic_slice with traced start keeps the program SPMD)
    s0 = (hf_flags[1] * T).astype(jnp.int32)
    sl = lambda x: jax.lax.dynamic_slice_in_dim(x, s0, T, axis=0)
    h = sl(h_all)                                         # (T, D)
    A_r = sl(A_r_all)
    A_m = sl(A_m_all)
    rvh = _mm(h, reg_val_w.T) + reg_val_b
    mvh = _mm(h, mem_val_w.T) + mem_val_b
    rqh = _mm(h, reg_q_w.T) + reg_q_b
    mqh = _mm(h, mem_q_w.T) + mem_q_b

    W_h, W_r, W_m = comb_w[:, :D], comb_w[:, D:2 * D], comb_w[:, 2 * D:]
    pre = _mm(h, W_h.T) + comb_b

    causal = (t[:, None] >= t[None, :]).astype(F32)
    dpow = jnp.exp((t + 1.0) * logd)[:, None]             # DECAY^(t+1)
    dp = jnp.exp(t * logd)[:, None]                       # DECAY^t
    dm = jnp.exp(-t * logd)[:, None]                      # DECAY^-s

    # register bank
    G = _mm(rqh, rvh.T)                                   # (T, T)
    scores = scale * (_mm(G * causal, A_r) + _mm(rqh, reg_init.T))
    P = sm(scores, axis=-1)                               # (T, NREG)
    R = _mm(_mm(P, A_r.T) * causal, rvh) + _mm(P, reg_init)

    # persistent memory: decay^(t-s) folded into row scalings
    Gm = _mm(mqh * dp, (mvh * dm).T)
    scores_m = scale * (_mm(Gm * causal, A_m) + dpow * _mm(mqh, mem_init.T))
    Pm = sm(scores_m, axis=-1)
    M = dp * _mm(_mm(Pm, (A_m * dm).T) * causal, mvh) + dpow * _mm(Pm, mem_init)

    c = pre + _mm(R, W_r.T) + _mm(M, W_m.T) + h
    mu = c.mean(-1, keepdims=True)
    v = ((c - mu) ** 2).mean(-1, keepdims=True)
    return ln_g * (c - mu) * jax.lax.rsqrt(v + 1e-5) + ln_b


_pmapped = None


def _get_pmapped():
    global _pmapped
    if _pmapped is None:
        _pmapped = jax.pmap(
            _shard_fn,
            in_axes=(0, 0) + (None,) * len(_W_NAMES),
            devices=jax.devices()[:8],
        )
    return _pmapped


def kernel(**inputs):
    u = np.asarray(inputs["u"], dtype=np.float32)
    ws = [np.asarray(inputs[k], dtype=np.float32) for k in _W_NAMES]

    # shard i -> (batch i//2, half i%2); each shard carries the full batch seq
    u_sh = np.stack([u[i // 2] for i in range(8)])               # (8, L, D)
    hf = np.array([[float(i % 2), float(i % 2)] for i in range(8)],
                  dtype=np.float32)                              # (8, 2)

    out_sh = _get_pmapped()(u_sh, hf, *ws)                       # (8, T, D)
    out_sh = np.asarray(jax.device_get(out_sh), dtype=np.float32)

    out = np.empty((B, L, D), dtype=np.float32)
    for b in range(B):
        out[b, :T] = out_sh[2 * b]
        out[b, T:] = out_sh[2 * b + 1]
    return out
